# revision 18
# baseline (speedup 1.0000x reference)
"""CTC loss on 8 Trainium2 NeuronCores (Bass/Tile).

Strategy (data parallel, per the sharding hint): batch B=64 is split 8
samples/core. The host gathers each sample's 31 distinct lattice emission
rows (1 blank + 30 labels) from log_probs — a 4MB slice of the 170MB
input — and ships only that to the devices, packed directly in the
(lane=(sample,chunk), slot, t') layout the kernel consumes. Each core runs
the CTC forward recurrence in linear space:

  - per-(sample,t) max normalization (emission planes exp'd on device),
  - lattice rows computed as first-order scans over t (tensor_tensor_scan),
  - T split into C=16 chunks mapped to SBUF partitions (lanes = (b, c)),
    cross-chunk carries solved exactly with per-slot transfer matrices G
    built on the PE/ACT from bulk chunk-sum cumulants,
  - per-(sample,chunk) static log offsets (host-estimated via a coarse
    windowed DP) keep all stored values in fp32 range; the stitch algebra
    folds the offsets in exactly, so they do not affect the result.

Per-sample losses are reconstructed on host from a tiny (3,128,33) output
per core (final two lattice rows + normalization cumsums): a final mean
over per-sample losses, as in the reference.
"""
import numpy as np

import concourse.bass as bass
import concourse.bacc as bacc
import concourse.tile as tile
from concourse import mybir
from concourse.bass_utils import run_bass_kernel_spmd

import jax
import jax.numpy as jnp
from jax import lax

F32 = mybir.dt.float32
BF16 = mybir.dt.bfloat16
I32 = mybir.dt.int32

T, B, V, S = 512, 64, 1296, 30
L = 2 * S + 1          # 61 lattice rows
NS = S + 1             # 31 distinct emission slots (slot 0 = blank)
NSP = 32               # padded slot count
C = 16                 # time chunks  (lanes = 8 local samples x 16 chunks)
TC = T // C            # 32 steps per chunk
NCORES = 8
BLOC = B // NCORES     # 8 samples per core
BLANK = 0
NEG = np.float32(-1e30)

_prog_cache = {}

_SLOTMAP = np.array([0 if l % 2 == 0 else (l + 1) // 2 for l in range(L)])


def _slot(l):
    return 0 if l % 2 == 0 else (l + 1) // 2


# --------------------------------------------------------------------------
# host-side prep
# --------------------------------------------------------------------------

_WIN = 2
_NW = T // _WIN

# column layout of the per-core f32 input blob [128, _BLOB_W]
_M0 = 0                       # m (TC)
_LAM0 = _M0 + TC              # lam (1)
_AL0 = _LAM0 + 1              # allow2 (29)
_E00 = _AL0 + 29              # e0 (TC)
_BLOB_W = _E00 + TC


def _make_prep_jit():
    cpu = jax.devices("cpu")[0]
    slotmap = jnp.asarray(_SLOTMAP)

    def _prep(em, t2):                 # em: (T, B, NS) f32; t2: (B, S) i32
        m = em.max(axis=2)             # (T, B)
        zw_ns = (em.reshape(_NW, _WIN, B, NS).sum(axis=1)
                 - m.reshape(_NW, _WIN, B).sum(axis=1)[:, :, None]) / _WIN
        zw = zw_ns[:, :, slotmap]      # (nw, B, L)
        v0 = jnp.full((B, L), NEG, jnp.float32).at[:, 0].set(0.0).at[:, 1].set(0.0)

        def step(v, zwi):
            for _ in range(_WIN):
                p1 = jnp.pad(v[:, :-1], ((0, 0), (1, 0)), constant_values=NEG)
                p2 = jnp.pad(v[:, :-2], ((0, 0), (2, 0)), constant_values=NEG)
                mx = jnp.maximum(jnp.maximum(v, p1), p2)
                s = (jnp.exp(v - mx) + jnp.exp(p1 - mx) + jnp.exp(p2 - mx))
                v = mx + jnp.log(s) + zwi
            return v, v.max(axis=1)

        _, lev = lax.scan(step, v0, zw)          # (nw, B)
        wpc = TC // _WIN
        Lam = lev[wpc // 2::wpc, :].T            # (B, C) chunk-middle levels

        # emission planes in device lane layout
        emis = jnp.zeros((B, C, NSP, TC), jnp.float32)
        emis = emis.at[:, :, :NS, :].set(
            em.reshape(C, TC, B, NS).transpose(2, 0, 3, 1))
        mlane = m.T.reshape(B, C, TC)

        # allow mask (skip-transition) per lattice odd row
        ext = jnp.zeros((B, L), jnp.int32).at[:, 1::2].set(t2)
        ext_m2 = jnp.pad(ext[:, :-2], ((0, 0), (2, 0)))
        allow = ((ext != BLANK) & (ext != ext_m2)).astype(jnp.float32)
        allow2 = allow[:, 3::2]                  # (B, 29)
        al_lane = jnp.broadcast_to(allow2[:, None, :], (B, C, 29))

        e0 = jnp.zeros((B, C, TC), jnp.float32).at[:, 0, 0].set(
            jnp.exp(-Lam[:, 0]))

        blob = jnp.concatenate([
            mlane.reshape(B * C, TC),
            Lam.reshape(B * C, 1),
            al_lane.reshape(B * C, 29),
            e0.reshape(B * C, TC),
        ], axis=1)                               # (1024, _BLOB_W)
        return emis.reshape(B * C, NSP * TC).astype(jnp.bfloat16), blob, Lam

    return jax.jit(_prep, device=cpu)


_prep_jit = None


def _host_prep(log_probs, targets):
    """Per-core input blobs (lane layout) + per-(b,chunk) offsets Lam."""
    global _prep_jit
    t2 = np.asarray(targets).reshape(B, S).astype(np.int64)
    vrows = np.zeros((B, NS), np.int64)
    vrows[:, 1:] = t2                      # slot s>=1 -> label s-1; slot 0 = blank

    # gather only the needed emission rows: em[t,b,s] = log_probs[t,b,vrows[b,s]]
    flat = log_probs.reshape(T, B * V)
    cols = (np.arange(B)[:, None] * V + vrows).ravel()
    em = flat[:, cols].reshape(T, B, NS)

    # level-estimate DP + blob packing, one XLA-CPU call
    if _prep_jit is None:
        _prep_jit = _make_prep_jit()
    emis, blob, Lam = _prep_jit(em, t2.astype(np.int32))
    return np.asarray(emis), np.asarray(blob), np.asarray(Lam)


def _static_mats():
    """Block tri matrices over lanes (b,c): same for every core."""
    bi = np.arange(128) // C
    ci = np.arange(128) % C
    same_b = bi[:, None] == bi[None, :]
    tric = (same_b & (ci[:, None] <= ci[None, :])).astype(np.float32)
    trics = (same_b & (ci[:, None] < ci[None, :])).astype(np.float32)
    tribias = np.where(trics > 0, np.float32(0.0), NEG).astype(np.float32)
    ident = np.eye(128, dtype=np.float32)
    return tric, trics, tribias, ident


# --------------------------------------------------------------------------
# device program (identical for all cores; per-core data differs)
# --------------------------------------------------------------------------

def _build_program():
    nc = bacc.Bacc(None)
    d_emis = nc.declare_dram_parameter("emis", [128, NSP, TC], BF16, isOutput=False)
    d_blob = nc.declare_dram_parameter("blob", [128, _BLOB_W], F32, isOutput=False)
    out = nc.declare_dram_parameter("out", [3, 128, TC + 1], F32, isOutput=True)

    with tile.TileContext(nc) as tc:
        with (
            tc.tile_pool(name="consts", bufs=1) as consts,
            tc.tile_pool(name="rows", bufs=1) as rowsp,
            tc.tile_pool(name="work", bufs=3) as work,
            tc.tile_pool(name="gpool", bufs=3) as gpool,
            tc.tile_pool(name="gamp", bufs=2) as gamp,
            tc.tile_pool(name="ps", bufs=2, space="PSUM") as ps,
            tc.tile_pool(name="ps1", bufs=1, space="PSUM") as ps1,
        ):
            # ---- static lane matrices, built on device ----
            # tric[p,j] = (p//16 == j//16) & (p <= j); cols decompose as
            # j = jb*16 + jc, so the block predicate is affine via the
            # 2D column pattern [[-16, 8], [0, 16]].
            sb_tric = consts.tile([128, 128], F32)
            nc.gpsimd.memset(sb_tric[:], 1.0)
            nc.gpsimd.affine_select(          # p - 16*jb >= 0
                out=sb_tric[:], in_=sb_tric[:], pattern=[[-16, 8], [0, 16]],
                channel_multiplier=1, base=0,
                compare_op=mybir.AluOpType.is_ge, fill=0.0)
            nc.gpsimd.affine_select(          # 15 + 16*jb - p >= 0
                out=sb_tric[:], in_=sb_tric[:], pattern=[[16, 8], [0, 16]],
                channel_multiplier=-1, base=15,
                compare_op=mybir.AluOpType.is_ge, fill=0.0)
            nc.gpsimd.affine_select(          # j - p >= 0
                out=sb_tric[:], in_=sb_tric[:], pattern=[[1, 128]],
                channel_multiplier=-1, base=0,
                compare_op=mybir.AluOpType.is_ge, fill=0.0)
            sb_ident = consts.tile([128, 128], F32)
            nc.gpsimd.memset(sb_ident[:], 1.0)
            nc.gpsimd.affine_select(          # j - p >= 0
                out=sb_ident[:], in_=sb_ident[:], pattern=[[1, 128]],
                channel_multiplier=-1, base=0,
                compare_op=mybir.AluOpType.is_ge, fill=0.0)
            nc.gpsimd.affine_select(          # p - j >= 0
                out=sb_ident[:], in_=sb_ident[:], pattern=[[-1, 128]],
                channel_multiplier=1, base=0,
                compare_op=mybir.AluOpType.is_ge, fill=0.0)
            sb_trics = consts.tile([128, 128], F32)
            nc.vector.tensor_tensor(out=sb_trics[:], in0=sb_tric[:],
                                    in1=sb_ident[:],
                                    op=mybir.AluOpType.subtract)
            sb_tribias = consts.tile([128, 128], F32)
            nc.vector.tensor_scalar(          # trics>0 -> 0 else NEG
                out=sb_tribias[:], in0=sb_trics[:], scalar1=1.0, scalar2=1e30,
                op0=mybir.AluOpType.subtract, op1=mybir.AluOpType.mult)

            # ---- const loads (column slices of the single f32 blob) ----
            sb_lam = consts.tile([128, 1], F32)
            nc.sync.dma_start(out=sb_lam[:], in_=d_blob[:, _LAM0:_LAM0 + 1])
            sb_allow2 = consts.tile([128, 29], F32)
            nc.sync.dma_start(out=sb_allow2[:], in_=d_blob[:, _AL0:_AL0 + 29])
            sb_e0 = consts.tile([128, TC], F32)
            nc.sync.dma_start(out=sb_e0[:], in_=d_blob[:, _E00:_E00 + TC])
            sb_ones = consts.tile([1, 128], F32)
            nc.vector.memset(sb_ones[:], 1.0)
            sb_zeros = consts.tile([128, TC], F32)
            nc.vector.memset(sb_zeros[:], 0.0)

            # ---- emission planes: host-gathered, lane layout, bf16 ----
            sb_lp16 = consts.tile([128, NSP, TC], BF16)
            nc.sync.dma_start(out=sb_lp16[:], in_=d_emis[:])
            sb_lp = consts.tile([128, NSP, TC], F32)
            nc.scalar.copy(sb_lp[:], sb_lp16[:])

            # ---- normalization / cumulants, in slot groups of 8 ----
            sb_m = consts.tile([128, TC], F32)
            nc.sync.dma_start(out=sb_m[:], in_=d_blob[:, _M0:_M0 + TC])
            cumM = consts.tile([128, TC], F32)
            nc.vector.tensor_tensor_scan(
                out=cumM[:], data0=sb_m[:], data1=sb_zeros[:], initial=0.0,
                op0=mybir.AluOpType.add, op1=mybir.AluOpType.add)
            ps_baseM = ps1.tile([128, 1], F32, tag="bulk")
            nc.tensor.matmul(out=ps_baseM[:], lhsT=sb_trics[:],
                             rhs=cumM[:, TC - 1:TC], start=True, stop=True)
            sb_baseM = consts.tile([128, 1], F32)
            nc.scalar.copy(sb_baseM[:], ps_baseM[:])

            sb_z = consts.tile([128, NS, TC], F32)
            sb_p = consts.tile([128, NS, TC], F32)
            sb_S = consts.tile([128, NS], F32)
            biasvec = consts.tile([128, NS], F32)
            msider = consts.tile([128, NS], F32)
            mb = sb_m[:]
            GRP = 8
            for g0 in range(0, NS, GRP):
                g1 = min(g0 + GRP, NS)
                n = g1 - g0
                m_bcast = bass.AP(tensor=mb.tensor, offset=mb.offset,
                                  ap=[mb.ap[0], [0, n], mb.ap[1]])
                nc.vector.tensor_tensor(out=sb_z[:, g0:g1, :],
                                        in0=sb_lp[:, g0:g1, :], in1=m_bcast,
                                        op=mybir.AluOpType.subtract)
                nc.vector.tensor_reduce(out=sb_S[:, g0:g1],
                                        in_=sb_z[:, g0:g1, :],
                                        axis=mybir.AxisListType.X,
                                        op=mybir.AluOpType.add)
                nc.scalar.activation(sb_p[:, g0:g1, :], sb_z[:, g0:g1, :],
                                     mybir.ActivationFunctionType.Exp)
                ps_lc = ps1.tile([128, GRP], F32, tag="bulk")
                nc.tensor.matmul(out=ps_lc[:, 0:n], lhsT=sb_tric[:],
                                 rhs=sb_S[:, g0:g1], start=True, stop=True)
                nc.vector.tensor_scalar(
                    out=biasvec[:, g0:g1], in0=ps_lc[:, 0:n], scalar1=-1.0,
                    scalar2=sb_lam[:],
                    op0=mybir.AluOpType.mult, op1=mybir.AluOpType.add)
                ps_lcs = ps1.tile([128, GRP], F32, tag="bulk2")
                nc.tensor.matmul(out=ps_lcs[:, 0:n], lhsT=sb_trics[:],
                                 rhs=sb_S[:, g0:g1], start=True, stop=True)
                nc.vector.tensor_scalar(
                    out=msider[:, g0:g1], in0=ps_lcs[:, 0:n],
                    scalar1=sb_lam[:], scalar2=None,
                    op0=mybir.AluOpType.subtract)

            # ---- per-slot G transfer matrices ----
            def build_G(s, pool, tag):
                ps_t = ps.tile([1, 128], F32, tag="ps_t")
                nc.tensor.transpose(out=ps_t[:], in_=msider[:, s:s + 1],
                                    identity=sb_ident[:])
                stg = work.tile([1, 128], F32, tag="stg")
                nc.scalar.copy(stg[:], ps_t[:])
                psG = ps.tile([128, 128], F32, tag="psG")
                nc.tensor.matmul(out=psG[:], lhsT=sb_ones[:],
                                 rhs=stg[:], start=True, stop=False)
                nc.tensor.matmul(out=psG[:], lhsT=sb_ident[:],
                                 rhs=sb_tribias[:], start=False, stop=True)
                Gt = pool.tile([128, 128], F32, tag=tag)
                nc.scalar.activation(Gt[:], psG[:],
                                     mybir.ActivationFunctionType.Exp,
                                     bias=biasvec[:, s:s + 1])
                return Gt

            G_blank = build_G(0, consts, "Gblank")

            # ---- lattice rows ----
            row_tiles = []
            gam_prev = {}
            for l in range(L):
                s = _slot(l)
                Gt = G_blank if s == 0 else build_G(s, gpool, "G")
                p_l = sb_p[:, s, :]
                if l == 0:
                    src_ap = sb_e0[:]
                elif l == 1:
                    srct = work.tile([128, TC], F32, tag="src")
                    nc.vector.tensor_add(out=srct[:],
                                         in0=row_tiles[0][:, 0:TC],
                                         in1=sb_e0[:])
                    src_ap = srct[:]
                elif l % 2 == 0:
                    src_ap = row_tiles[l - 1][:, 0:TC]
                else:
                    srct = work.tile([128, TC], F32, tag="src")
                    nc.vector.tensor_add(out=srct[:],
                                         in0=row_tiles[l - 1][:, 0:TC],
                                         in1=gam_prev[l - 2][:, 0:TC])
                    src_ap = srct[:]

                loc = work.tile([128, TC], F32, tag="loc")
                nc.vector.tensor_tensor_scan(
                    out=loc[:], data0=src_ap, data1=p_l, initial=0.0,
                    op0=mybir.AluOpType.add, op1=mybir.AluOpType.mult)
                xps = ps.tile([128, 1], F32, tag="xps")
                nc.tensor.matmul(out=xps[:], lhsT=Gt[:],
                                 rhs=loc[:, TC - 1:TC], start=True, stop=True)
                rowl = rowsp.tile([128, TC + 1], F32, tag=f"row{l}")
                nc.vector.tensor_tensor_scan(
                    out=rowl[:, 1:TC + 1], data0=src_ap, data1=p_l,
                    initial=xps[:, 0:1],
                    op0=mybir.AluOpType.add, op1=mybir.AluOpType.mult)
                nc.scalar.copy(rowl[:, 0:1], xps[:, 0:1])
                row_tiles.append(rowl)
                if l % 2 == 1 and l + 2 < L:
                    gaml = gamp.tile([128, TC + 1], F32, tag="gam")
                    nc.scalar.mul(gaml[:], rowl[:],
                                  sb_allow2[:, (l - 1) // 2:(l - 1) // 2 + 1])
                    gam_prev[l] = gaml

            # ---- outputs ----
            nc.sync.dma_start(out=out[0], in_=row_tiles[L - 2][:])
            nc.sync.dma_start(out=out[1], in_=row_tiles[L - 1][:])
            nc.sync.dma_start(out=out[2, :, 1:TC + 1], in_=cumM[:])
            nc.sync.dma_start(out=out[2, :, 0:1], in_=sb_baseM[:])
    nc.finalize()
    return nc


# --------------------------------------------------------------------------
# entry point
# --------------------------------------------------------------------------

def kernel(log_probs, targets, input_lengths, target_lengths):
    log_probs = np.ascontiguousarray(np.asarray(log_probs, dtype=np.float32))
    targets = np.asarray(targets)
    input_lengths = np.asarray(input_lengths).astype(np.int64)
    target_lengths = np.asarray(target_lengths)

    emis, blob, Lam = _host_prep(log_probs, targets)

    if "nc" not in _prog_cache:
        _prog_cache["nc"] = _build_program()
    nc = _prog_cache["nc"]

    in_maps = [{"emis": emis[k * 128:(k + 1) * 128].reshape(128, NSP, TC),
                "blob": blob[k * 128:(k + 1) * 128]} for k in range(NCORES)]

    res = run_bass_kernel_spmd(nc, in_maps, core_ids=list(range(NCORES)))

    # host-side: per-sample loss extraction + mean (the "all-reduce")
    losses = np.zeros(B, np.float64)
    tE = input_lengths - 1
    cb, tb = tE // TC, tE % TC
    for k in range(NCORES):
        o = res.results[k]["out"]              # (3, 128, TC+1)
        for b in range(BLOC):
            gb = k * BLOC + b
            lane = b * C + cb[gb]
            A = np.float64(o[0, lane, 1 + tb[gb]]) + np.float64(o[1, lane, 1 + tb[gb]])
            lnorm = (np.float64(o[2, lane, 0]) + np.float64(o[2, lane, 1 + tb[gb]])
                     + np.float64(Lam[gb, cb[gb]]))
            lb = -(np.log(A) + lnorm) if A > 0 else np.inf
            if not np.isfinite(lb) or lb >= 1e29:
                lb = 0.0
            losses[gb] = lb
    result = np.float32(np.mean((losses / target_lengths.astype(np.float64))
                                .astype(np.float32)))
    return np.asarray(result, dtype=np.float32)


# revision 23
# speedup vs baseline: 2.2636x; 2.2636x over previous
"""CTC loss on 8 Trainium2 NeuronCores (Bass/Tile).

Strategy (data parallel, per the sharding hint): batch B=64 is split 8
samples/core. The host gathers each sample's 31 distinct lattice emission
rows (1 blank + 30 labels) from log_probs — a 4MB slice of the 170MB
input — and ships only that to the devices, packed directly in the
(lane=(sample,chunk), slot, t') layout the kernel consumes. Each core runs
the CTC forward recurrence in linear space:

  - per-(sample,t) max normalization (emission planes exp'd on device),
  - lattice rows computed as first-order scans over t (tensor_tensor_scan),
  - T split into C=16 chunks mapped to SBUF partitions (lanes = (b, c)),
    cross-chunk carries solved exactly with per-slot transfer matrices G
    built on the PE/ACT from bulk chunk-sum cumulants,
  - per-(sample,chunk) static log offsets (host-estimated via a coarse
    windowed DP) keep all stored values in fp32 range; the stitch algebra
    folds the offsets in exactly, so they do not affect the result.

Per-sample losses are reconstructed on host from a tiny (3,128,33) output
per core (final two lattice rows + normalization cumsums): a final mean
over per-sample losses, as in the reference.
"""
import numpy as np

import concourse.bass as bass
import concourse.bacc as bacc
import concourse.tile as tile
from concourse import mybir
from concourse.bass_utils import run_bass_kernel_spmd

import jax
import jax.numpy as jnp
from jax import lax

try:
    jax.config.update("jax_compilation_cache_dir", "/tmp/jax_persist_cache")
    jax.config.update("jax_persistent_cache_min_compile_time_secs", 0.0)
except Exception:
    pass

F32 = mybir.dt.float32
BF16 = mybir.dt.bfloat16
I32 = mybir.dt.int32

T, B, V, S = 512, 64, 1296, 30
L = 2 * S + 1          # 61 lattice rows
NS = S + 1             # 31 distinct emission slots (slot 0 = blank)
NSP = 32               # padded slot count
C = 16                 # time chunks  (lanes = 8 local samples x 16 chunks)
TC = T // C            # 32 steps per chunk
NCORES = 8
BLOC = B // NCORES     # 8 samples per core
BLANK = 0
NEG = np.float32(-1e30)

_prog_cache = {}

_SLOTMAP = np.array([0 if l % 2 == 0 else (l + 1) // 2 for l in range(L)])


def _slot(l):
    return 0 if l % 2 == 0 else (l + 1) // 2


# --------------------------------------------------------------------------
# host-side prep
# --------------------------------------------------------------------------

_WIN = 2               # level-DP window (1 logsumexp application per window)
_NW = T // _WIN

# column layout of the per-core f32 input blob [128, _BLOB_W]
_M0 = 0                       # m (TC)
_LAM0 = _M0 + TC              # lam (1)
_AL0 = _LAM0 + 1              # allow2 (29)
_E00 = _AL0 + 29              # e0 (TC)
_BLOB_W = _E00 + TC


def _make_prep_jit():
    cpu = jax.devices("cpu")[0]
    slotmap = jnp.asarray(_SLOTMAP)

    def _prep(em, t2):                 # em: (T, B, NS) f32; t2: (B, S) i32
        m = em.max(axis=2)             # (T, B)
        # window-SUM of z (one DP application tracks the window's full
        # emission mass; lattice advance <=2 per window is still ample)
        zw_ns = (em.reshape(_NW, _WIN, B, NS).sum(axis=1)
                 - m.reshape(_NW, _WIN, B).sum(axis=1)[:, :, None])
        zw = zw_ns[:, :, slotmap]      # (nw, B, L)
        v0 = jnp.full((B, L), NEG, jnp.float32).at[:, 0].set(0.0).at[:, 1].set(0.0)

        def step(v, zwi):
            p1 = jnp.pad(v[:, :-1], ((0, 0), (1, 0)), constant_values=NEG)
            p2 = jnp.pad(v[:, :-2], ((0, 0), (2, 0)), constant_values=NEG)
            mx = jnp.maximum(jnp.maximum(v, p1), p2)
            s = (jnp.exp(v - mx) + jnp.exp(p1 - mx) + jnp.exp(p2 - mx))
            v = mx + jnp.log(s) + zwi
            return v, v.max(axis=1)

        _, lev = lax.scan(step, v0, zw)          # (nw, B)
        wpc = TC // _WIN
        # chunk-middle levels; +16 recenters the one-app DP's systematic
        # underestimate (about -5..-39 nats vs the exact two-app DP)
        Lam = lev[wpc // 2::wpc, :].T + 16.0     # (B, C)

        # emission planes in device lane layout
        emis = jnp.zeros((B, C, NSP, TC), jnp.float32)
        emis = emis.at[:, :, :NS, :].set(
            em.reshape(C, TC, B, NS).transpose(2, 0, 3, 1))
        mlane = m.T.reshape(B, C, TC)

        # allow mask (skip-transition) per lattice odd row
        ext = jnp.zeros((B, L), jnp.int32).at[:, 1::2].set(t2)
        ext_m2 = jnp.pad(ext[:, :-2], ((0, 0), (2, 0)))
        allow = ((ext != BLANK) & (ext != ext_m2)).astype(jnp.float32)
        allow2 = allow[:, 3::2]                  # (B, 29)
        al_lane = jnp.broadcast_to(allow2[:, None, :], (B, C, 29))

        e0 = jnp.zeros((B, C, TC), jnp.float32).at[:, 0, 0].set(
            jnp.exp(-Lam[:, 0]))

        blob = jnp.concatenate([
            mlane.reshape(B * C, TC),
            Lam.reshape(B * C, 1),
            al_lane.reshape(B * C, 29),
            e0.reshape(B * C, TC),
        ], axis=1)                               # (1024, _BLOB_W)
        return emis.reshape(B * C, NSP * TC).astype(jnp.bfloat16), blob, Lam

    return jax.jit(_prep, device=cpu)


_prep_jit = None


def _host_prep(log_probs, targets):
    """Per-core input blobs (lane layout) + per-(b,chunk) offsets Lam."""
    global _prep_jit
    t2 = np.asarray(targets).reshape(B, S).astype(np.int64)
    vrows = np.zeros((B, NS), np.int64)
    vrows[:, 1:] = t2                      # slot s>=1 -> label s-1; slot 0 = blank

    # gather only the needed emission rows: em[t,b,s] = log_probs[t,b,vrows[b,s]]
    flat = log_probs.reshape(T, B * V)
    cols = (np.arange(B)[:, None] * V + vrows).ravel()
    em = flat[:, cols].reshape(T, B, NS)

    # level-estimate DP + blob packing, one XLA-CPU call
    if _prep_jit is None:
        _prep_jit = _make_prep_jit()
    emis, blob, Lam = _prep_jit(em, t2.astype(np.int32))
    return np.asarray(emis), np.asarray(blob), np.asarray(Lam)


def _static_mats():
    """Block tri matrices over lanes (b,c): same for every core."""
    bi = np.arange(128) // C
    ci = np.arange(128) % C
    same_b = bi[:, None] == bi[None, :]
    tric = (same_b & (ci[:, None] <= ci[None, :])).astype(np.float32)
    trics = (same_b & (ci[:, None] < ci[None, :])).astype(np.float32)
    tribias = np.where(trics > 0, np.float32(0.0), NEG).astype(np.float32)
    ident = np.eye(128, dtype=np.float32)
    return tric, trics, tribias, ident


# --------------------------------------------------------------------------
# device program (identical for all cores; per-core data differs)
# --------------------------------------------------------------------------

def _build_program():
    nc = bacc.Bacc(None)
    d_emis = nc.declare_dram_parameter("emis", [128, NSP, TC], BF16, isOutput=False)
    d_blob = nc.declare_dram_parameter("blob", [128, _BLOB_W], F32, isOutput=False)
    out = nc.declare_dram_parameter("out", [3, 128, TC + 1], F32, isOutput=True)

    with tile.TileContext(nc) as tc:
        with (
            tc.tile_pool(name="consts", bufs=1) as consts,
            tc.tile_pool(name="rows", bufs=1) as rowsp,
            tc.tile_pool(name="work", bufs=3) as work,
            tc.tile_pool(name="gpool", bufs=3) as gpool,
            tc.tile_pool(name="gamp", bufs=2) as gamp,
            tc.tile_pool(name="ps", bufs=2, space="PSUM") as ps,
            tc.tile_pool(name="ps1", bufs=1, space="PSUM") as ps1,
        ):
            # ---- static lane matrices, built on device ----
            # tric[p,j] = (p//16 == j//16) & (p <= j); cols decompose as
            # j = jb*16 + jc, so the block predicate is affine via the
            # 2D column pattern [[-16, 8], [0, 16]].
            sb_tric = consts.tile([128, 128], F32)
            nc.gpsimd.memset(sb_tric[:], 1.0)
            nc.gpsimd.affine_select(          # p - 16*jb >= 0
                out=sb_tric[:], in_=sb_tric[:], pattern=[[-16, 8], [0, 16]],
                channel_multiplier=1, base=0,
                compare_op=mybir.AluOpType.is_ge, fill=0.0)
            nc.gpsimd.affine_select(          # 15 + 16*jb - p >= 0
                out=sb_tric[:], in_=sb_tric[:], pattern=[[16, 8], [0, 16]],
                channel_multiplier=-1, base=15,
                compare_op=mybir.AluOpType.is_ge, fill=0.0)
            nc.gpsimd.affine_select(          # j - p >= 0
                out=sb_tric[:], in_=sb_tric[:], pattern=[[1, 128]],
                channel_multiplier=-1, base=0,
                compare_op=mybir.AluOpType.is_ge, fill=0.0)
            sb_ident = consts.tile([128, 128], F32)
            nc.gpsimd.memset(sb_ident[:], 1.0)
            nc.gpsimd.affine_select(          # j - p >= 0
                out=sb_ident[:], in_=sb_ident[:], pattern=[[1, 128]],
                channel_multiplier=-1, base=0,
                compare_op=mybir.AluOpType.is_ge, fill=0.0)
            nc.gpsimd.affine_select(          # p - j >= 0
                out=sb_ident[:], in_=sb_ident[:], pattern=[[-1, 128]],
                channel_multiplier=1, base=0,
                compare_op=mybir.AluOpType.is_ge, fill=0.0)
            sb_trics = consts.tile([128, 128], F32)
            nc.vector.tensor_tensor(out=sb_trics[:], in0=sb_tric[:],
                                    in1=sb_ident[:],
                                    op=mybir.AluOpType.subtract)
            sb_tribias = consts.tile([128, 128], F32)
            nc.vector.tensor_scalar(          # trics>0 -> 0 else NEG
                out=sb_tribias[:], in0=sb_trics[:], scalar1=1.0, scalar2=1e30,
                op0=mybir.AluOpType.subtract, op1=mybir.AluOpType.mult)

            # ---- const loads (column slices of the single f32 blob) ----
            sb_lam = consts.tile([128, 1], F32)
            nc.sync.dma_start(out=sb_lam[:], in_=d_blob[:, _LAM0:_LAM0 + 1])
            sb_allow2 = consts.tile([128, 29], F32)
            nc.sync.dma_start(out=sb_allow2[:], in_=d_blob[:, _AL0:_AL0 + 29])
            sb_e0 = consts.tile([128, TC], F32)
            nc.sync.dma_start(out=sb_e0[:], in_=d_blob[:, _E00:_E00 + TC])
            sb_ones = consts.tile([1, 128], F32)
            nc.vector.memset(sb_ones[:], 1.0)
            sb_zeros = consts.tile([128, TC], F32)
            nc.vector.memset(sb_zeros[:], 0.0)

            # ---- emission planes: host-gathered, lane layout, bf16 ----
            sb_lp16 = consts.tile([128, NSP, TC], BF16)
            nc.sync.dma_start(out=sb_lp16[:], in_=d_emis[:])
            sb_lp = consts.tile([128, NSP, TC], F32)
            nc.scalar.copy(sb_lp[:], sb_lp16[:])

            # ---- normalization / cumulants, in slot groups of 8 ----
            sb_m = consts.tile([128, TC], F32)
            nc.sync.dma_start(out=sb_m[:], in_=d_blob[:, _M0:_M0 + TC])
            cumM = consts.tile([128, TC], F32)
            nc.vector.tensor_tensor_scan(
                out=cumM[:], data0=sb_m[:], data1=sb_zeros[:], initial=0.0,
                op0=mybir.AluOpType.add, op1=mybir.AluOpType.add)
            ps_baseM = ps1.tile([128, 1], F32, tag="bulk")
            nc.tensor.matmul(out=ps_baseM[:], lhsT=sb_trics[:],
                             rhs=cumM[:, TC - 1:TC], start=True, stop=True)
            sb_baseM = consts.tile([128, 1], F32)
            nc.scalar.copy(sb_baseM[:], ps_baseM[:])

            sb_z = consts.tile([128, NS, TC], F32)
            sb_p = consts.tile([128, NS, TC], F32)
            sb_S = consts.tile([128, NS], F32)
            biasvec = consts.tile([128, NS], F32)
            msider = consts.tile([128, NS], F32)
            mb = sb_m[:]
            GRP = 8
            for g0 in range(0, NS, GRP):
                g1 = min(g0 + GRP, NS)
                n = g1 - g0
                m_bcast = bass.AP(tensor=mb.tensor, offset=mb.offset,
                                  ap=[mb.ap[0], [0, n], mb.ap[1]])
                nc.vector.tensor_tensor(out=sb_z[:, g0:g1, :],
                                        in0=sb_lp[:, g0:g1, :], in1=m_bcast,
                                        op=mybir.AluOpType.subtract)
                nc.vector.tensor_reduce(out=sb_S[:, g0:g1],
                                        in_=sb_z[:, g0:g1, :],
                                        axis=mybir.AxisListType.X,
                                        op=mybir.AluOpType.add)
                nc.scalar.activation(sb_p[:, g0:g1, :], sb_z[:, g0:g1, :],
                                     mybir.ActivationFunctionType.Exp)
                ps_lc = ps1.tile([128, GRP], F32, tag="bulk")
                nc.tensor.matmul(out=ps_lc[:, 0:n], lhsT=sb_tric[:],
                                 rhs=sb_S[:, g0:g1], start=True, stop=True)
                nc.vector.tensor_scalar(
                    out=biasvec[:, g0:g1], in0=ps_lc[:, 0:n], scalar1=-1.0,
                    scalar2=sb_lam[:],
                    op0=mybir.AluOpType.mult, op1=mybir.AluOpType.add)
                ps_lcs = ps1.tile([128, GRP], F32, tag="bulk2")
                nc.tensor.matmul(out=ps_lcs[:, 0:n], lhsT=sb_trics[:],
                                 rhs=sb_S[:, g0:g1], start=True, stop=True)
                nc.vector.tensor_scalar(
                    out=msider[:, g0:g1], in0=ps_lcs[:, 0:n],
                    scalar1=sb_lam[:], scalar2=None,
                    op0=mybir.AluOpType.subtract)

            # ---- per-slot G transfer matrices ----
            def build_G(s, pool, tag):
                ps_t = ps.tile([1, 128], F32, tag="ps_t")
                nc.tensor.transpose(out=ps_t[:], in_=msider[:, s:s + 1],
                                    identity=sb_ident[:])
                stg = work.tile([1, 128], F32, tag="stg")
                nc.scalar.copy(stg[:], ps_t[:])
                psG = ps.tile([128, 128], F32, tag="psG")
                nc.tensor.matmul(out=psG[:], lhsT=sb_ones[:],
                                 rhs=stg[:], start=True, stop=False)
                nc.tensor.matmul(out=psG[:], lhsT=sb_ident[:],
                                 rhs=sb_tribias[:], start=False, stop=True)
                Gt = pool.tile([128, 128], F32, tag=tag)
                nc.scalar.activation(Gt[:], psG[:],
                                     mybir.ActivationFunctionType.Exp,
                                     bias=biasvec[:, s:s + 1])
                return Gt

            G_blank = build_G(0, consts, "Gblank")

            # ---- lattice rows ----
            row_tiles = []
            gam_prev = {}
            for l in range(L):
                s = _slot(l)
                Gt = G_blank if s == 0 else build_G(s, gpool, "G")
                p_l = sb_p[:, s, :]
                if l == 0:
                    src_ap = sb_e0[:]
                elif l == 1:
                    srct = work.tile([128, TC], F32, tag="src")
                    nc.vector.tensor_add(out=srct[:],
                                         in0=row_tiles[0][:, 0:TC],
                                         in1=sb_e0[:])
                    src_ap = srct[:]
                elif l % 2 == 0:
                    src_ap = row_tiles[l - 1][:, 0:TC]
                else:
                    srct = work.tile([128, TC], F32, tag="src")
                    nc.vector.tensor_add(out=srct[:],
                                         in0=row_tiles[l - 1][:, 0:TC],
                                         in1=gam_prev[l - 2][:, 0:TC])
                    src_ap = srct[:]

                loc = work.tile([128, TC], F32, tag="loc")
                nc.vector.tensor_tensor_scan(
                    out=loc[:], data0=src_ap, data1=p_l, initial=0.0,
                    op0=mybir.AluOpType.add, op1=mybir.AluOpType.mult)
                xps = ps.tile([128, 1], F32, tag="xps")
                nc.tensor.matmul(out=xps[:], lhsT=Gt[:],
                                 rhs=loc[:, TC - 1:TC], start=True, stop=True)
                rowl = rowsp.tile([128, TC + 1], F32, tag=f"row{l}")
                nc.vector.tensor_tensor_scan(
                    out=rowl[:, 1:TC + 1], data0=src_ap, data1=p_l,
                    initial=xps[:, 0:1],
                    op0=mybir.AluOpType.add, op1=mybir.AluOpType.mult)
                nc.scalar.copy(rowl[:, 0:1], xps[:, 0:1])
                row_tiles.append(rowl)
                if l % 2 == 1 and l + 2 < L:
                    gaml = gamp.tile([128, TC + 1], F32, tag="gam")
                    nc.scalar.mul(gaml[:], rowl[:],
                                  sb_allow2[:, (l - 1) // 2:(l - 1) // 2 + 1])
                    gam_prev[l] = gaml

            # ---- outputs ----
            nc.sync.dma_start(out=out[0], in_=row_tiles[L - 2][:])
            nc.sync.dma_start(out=out[1], in_=row_tiles[L - 1][:])
            nc.sync.dma_start(out=out[2, :, 1:TC + 1], in_=cumM[:])
            nc.sync.dma_start(out=out[2, :, 0:1], in_=sb_baseM[:])
    nc.finalize()
    return nc


# --------------------------------------------------------------------------
# entry point
# --------------------------------------------------------------------------

def kernel(log_probs, targets, input_lengths, target_lengths):
    log_probs = np.ascontiguousarray(np.asarray(log_probs, dtype=np.float32))
    targets = np.asarray(targets)
    input_lengths = np.asarray(input_lengths).astype(np.int64)
    target_lengths = np.asarray(target_lengths)

    emis, blob, Lam = _host_prep(log_probs, targets)

    if "nc" not in _prog_cache:
        _prog_cache["nc"] = _build_program()
    nc = _prog_cache["nc"]

    in_maps = [{"emis": emis[k * 128:(k + 1) * 128].reshape(128, NSP, TC),
                "blob": blob[k * 128:(k + 1) * 128]} for k in range(NCORES)]

    res = run_bass_kernel_spmd(nc, in_maps, core_ids=list(range(NCORES)))

    # host-side: per-sample loss extraction + mean (the "all-reduce")
    losses = np.zeros(B, np.float64)
    tE = input_lengths - 1
    cb, tb = tE // TC, tE % TC
    for k in range(NCORES):
        o = res.results[k]["out"]              # (3, 128, TC+1)
        for b in range(BLOC):
            gb = k * BLOC + b
            lane = b * C + cb[gb]
            A = np.float64(o[0, lane, 1 + tb[gb]]) + np.float64(o[1, lane, 1 + tb[gb]])
            lnorm = (np.float64(o[2, lane, 0]) + np.float64(o[2, lane, 1 + tb[gb]])
                     + np.float64(Lam[gb, cb[gb]]))
            lb = -(np.log(A) + lnorm) if A > 0 else np.inf
            if not np.isfinite(lb) or lb >= 1e29:
                lb = 0.0
            losses[gb] = lb
    result = np.float32(np.mean((losses / target_lengths.astype(np.float64))
                                .astype(np.float32)))
    return np.asarray(result, dtype=np.float32)
